# revision 28
# baseline (speedup 1.0000x reference)
"""Trainium2 Bass kernel for CausalSelfAttention (GQA + QK-RMSNorm + RoPE).

Problem shapes (hardcoded): B=2, T=2048, C=2048, n_head=16, n_kv_head=4,
head_dim=128. 8 NeuronCores: data-parallel over batch (2) x tensor-parallel
over kv-head groups (4). Core j handles batch j//4 and kv-head group j%4
(q heads 4*(j%4)..4*(j%4)+3). Each core computes a partial output projection
with its Wo row-slice; the host sums the 4 partials per batch (the unshard
step) and adds the bias.

On-device dtype is fp16 (inputs pre-cast on host) with fp32 PSUM accumulation.
Math layout (all matmuls contract over the partition dim):
  x^T from DRAM -> q/k/v projections in natural [t, d] layout -> RoPE on DVE
  (commutes with the per-(t,head) rms scale) -> RMSNorm scale via quake-rsqrt
  computed on DVE (bitcast + 2 Newton steps; avoids ACT table churn) ->
  DMA-xbar transpose to q^T/k^T -> S^T = k^T' @ q^T per tk-chunk into fp32
  PSUM -> exp on ACT (bias -16*ln2; uniform scale cancels in the softmax
  normalization) -> causal 0/1 mask multiply on DVE for diagonal blocks ->
  y_aug = P^T' @ [v | 1] accumulated over tk (ones column = softmax
  denominators) -> divide via DVE -> DMA-xbar transpose y -> out = y^T' @ Wo.

Engine/queue assignment: ACT runs ONLY exp (its FIFO is the attention
critical path). Sync (SP) issues all xbar transposes + output stores;
GPSIMD (SWDGE) issues steady-state bulk loads; the prologue-critical
x/weight loads are split between sync and scalar (scalar is idle until
the first exp ~40us in).

Emission schedule: prologue projects spans 0 AND 1 so the PE FIFO has
span-1 matmuls to chew on while the DVE rope/rsqrt chain for span 0
drains. Steady state weaves, per attention k-step (S matmul -> exp ->
PV matmuls of the previous k-step), one "filler" PE unit popped from a
per-span queue of projection (span s+2) and out-projection (span s-1)
matmuls, so the PE never head-of-line blocks on ACT.

PSUM (8 banks): tag a = 2 banks rotating {q_ps, kv_ps, o_ps};
tag s = 2 banks for S^T; tag y = 4 banks (one per concurrent PV
accumulation group).
"""
import sys
import numpy as np

for _p in ("/opt/trn_rl_repo",):
    if _p not in sys.path:
        sys.path.insert(0, _p)

import concourse.bass as bass
import concourse.bacc as bacc
import concourse.mybir as mybir
import concourse.tile as tile
from concourse.bass_utils import run_bass_kernel_spmd

F32 = mybir.dt.float32
F16 = mybir.dt.float16
I32 = mybir.dt.int32
AF = mybir.ActivationFunctionType
ALU = mybir.AluOpType

B, T, C = 2, 2048, 2048
N_HEAD, N_KV_HEAD = 16, 4
HD = 128           # head dim
HALF = 64
G = N_HEAD // N_KV_HEAD      # 4 q heads per kv head = heads per core
NC_ = 8
TP = 4                       # tensor-parallel width (kv heads)
QCOLS = G * HD               # 512 q columns per core
SCALE = 1.0 / float(np.sqrt(HD))
EXP_BIAS = -16.0 * float(np.log(2.0))   # keep exp() outputs inside fp16 range
EPS = float(np.finfo(np.float32).eps)
QUAKE = 0x5F3759DF

_CACHE = {}


def build_nc(t=T):
    """Build the SPMD Tile kernel for sequence length t (t % 512 == 0)."""
    nt = t // 128          # number of 128-row T-chunks
    nb = t // 512          # number of 512-wide tq blocks (spans)
    ncc = C // 128         # number of 128 C-chunks

    nc = bacc.Bacc("TRN2", target_bir_lowering=False, debug=False,
                   num_devices=NC_)

    nb_ = t // 512
    # all bulk inputs host-packed partition-major so every load is a
    # contiguous-per-partition DMA (fat descriptors, cheap HWDGE issue)
    xtd = nc.dram_tensor("xp", [nb_, 128, (C // 128) * 512], F16,
                         kind="ExternalInput")
    wq = nc.dram_tensor("wqp", [128, (C // 128) * QCOLS], F16,
                        kind="ExternalInput")
    wkv = nc.dram_tensor("wkvp", [128, (C // 128) * 2 * HD], F16,
                         kind="ExternalInput")
    wo = nc.dram_tensor("wop", [128, G * C], F16, kind="ExternalInput")
    cs5 = nc.dram_tensor("cosp", [128, (t // 128) * 5 * HD], F16,
                         kind="ExternalInput")
    sn5 = nc.dram_tensor("sinp", [128, (t // 128) * 5 * HD], F16,
                         kind="ExternalInput")
    maskd = nc.dram_tensor("mask01", [128, 128], F16, kind="ExternalInput")
    out = nc.dram_tensor("out", [t, C], F16, kind="ExternalOutput")

    with tile.TileContext(nc) as tc:
        with (
            tc.tile_pool(name="const", bufs=1) as cpool,
            tc.tile_pool(name="wts", bufs=1) as wpool,
            tc.tile_pool(name="big", bufs=1) as bpool,
            tc.tile_pool(name="xsp", bufs=2) as xsp,
            tc.tile_pool(name="ytb", bufs=2) as ypool,
            tc.tile_pool(name="ptp", bufs=4) as ppool,
            tc.tile_pool(name="work", bufs=3) as work,
            tc.tile_pool(name="ps", bufs=2, space="PSUM") as ps,
        ):
            # ---------------- persistent SBUF tensors ----------------------
            wq_t = wpool.tile([128, ncc, QCOLS], F16, tag="wq")
            wkv_t = wpool.tile([128, ncc, 2 * HD], F16, tag="wkv")
            cos_t = wpool.tile([128, nt, 5 * HD], F16, tag="cos")
            sin_t = wpool.tile([128, nt, 5 * HD], F16, tag="sin")
            wo_t = wpool.tile([128, G, C], F16, tag="wo")
            mask_t = cpool.tile([128, 128], F16, tag="mask")
            b_exp = cpool.tile([128, 1], F32, tag="bexp")

            qkT = bpool.tile([128, 5, t], F16, tag="qkT")
            v_aug = bpool.tile([128, nt * (HD + 1)], F16, tag="vaug")
            sumsq = bpool.tile([128, nt * 5], F32, tag="ssq")
            r_all = bpool.tile([128, nt * 5], F32, tag="rall")
            rwk = bpool.tile([128, 4, 5], F32, tag="rwk")   # quake scratch
            x_spans = [None, None]      # ping-pong [128, ncc, 512] tiles
            yT = [None, None]           # ping-pong [128, G, 512] tiles
            qk_nat = [None] * nt        # raw projections per chunk
            qk_r = [None] * nt          # roped + scaled per chunk

            # ---------------- DMA emission helpers --------------------------
            def load_x_span(s, eng, pieces=2):
                xt = xsp.tile([128, ncc, 512], F16, tag="x", name="x")
                x_spans[s % 2] = xt
                step = ncc // pieces
                for q in range(pieces):
                    sl = slice(q * step, (q + 1) * step)
                    eng.dma_start(xt[:, sl, :],
                                  xtd[s, :, q * step * 512:
                                      (q + 1) * step * 512])

            # ---------------- stage A: projection of one 128-chunk ---------
            def proj_mms(i, cc, q_ps, kv_ps, xt):
                """One contraction step (2 matmuls) of chunk i."""
                ii = i % 4
                st, spf = (cc == 0), (cc == ncc - 1)
                xa = xt[:, cc, ii * 128:(ii + 1) * 128]
                nc.tensor.matmul(q_ps[:], xa, wq_t[:, cc, :],
                                 start=st, stop=spf)
                nc.tensor.matmul(kv_ps[:], xa, wkv_t[:, cc, :],
                                 start=st, stop=spf)

            def proj_evac(i, q_ps, kv_ps):
                """fp16 evacuation + sumsq (DVE) + quake-r (GPSIMD)."""
                qn = work.tile([128, 5 * HD], F16, tag="qn", name="qn")
                qk_nat[i] = qn
                nc.vector.tensor_copy(qn[:, 0:QCOLS], q_ps[:])
                nc.vector.tensor_copy(qn[:, QCOLS:5 * HD], kv_ps[:, 0:HD])
                nc.vector.tensor_copy(
                    v_aug[:, i * (HD + 1):i * (HD + 1) + HD],
                    kv_ps[:, HD:2 * HD])
                # sumsq for 4 q heads + k head on DVE
                sq_scr = work.tile([128, 128], F16, tag="sqscr")
                for h in range(5):
                    nc.vector.scalar_tensor_tensor(
                        sq_scr[:], qn[:, h * HD:(h + 1) * HD], 1.0,
                        qn[:, h * HD:(h + 1) * HD], ALU.mult, ALU.mult,
                        accum_out=sumsq[:, i * 5 + h:i * 5 + h + 1])
                # r = rsqrt(sumsq/HD + eps) via quake + 2 Newton steps (DVE)
                c0, c1 = i * 5, i * 5 + 5
                a = rwk[:, i % 4, :]
                nc.vector.tensor_scalar(a, sumsq[:, c0:c1], 1.0 / HD, EPS,
                                        ALU.mult, ALU.add)
                r = r_all[:, c0:c1]
                ri = r.bitcast(I32)
                nc.vector.tensor_scalar(ri, a.bitcast(I32), 1, None,
                                        ALU.logical_shift_right)
                # QUAKE - y == (y - QUAKE) * -1; int arith runs through the
                # fp32 pipe (low bits round), fine for a Newton seed
                nc.vector.tensor_scalar(ri, ri, QUAKE, -1,
                                        ALU.subtract, ALU.mult)
                # Newton: r <- r * (1.5 - 0.5*a*r*r), twice
                for _ in range(2):
                    m2 = work.tile([128, 5], F32, tag="nrt")
                    nc.vector.tensor_tensor(m2[:], r, r, ALU.mult)
                    nc.vector.tensor_tensor(m2[:], m2[:], a, ALU.mult)
                    nc.vector.tensor_scalar(m2[:], m2[:], -0.5, 1.5,
                                            ALU.mult, ALU.add)
                    nc.vector.tensor_tensor(r, r, m2[:], ALU.mult)

            # ---------------- stage A1b: rope + scale + transpose ----------
            def finish_chunk(i):
                """RoPE + rms scale + DMA-xbar transpose chunk i.

                c1/c2 are host-interleaved so the two rope products are
                single full-width contiguous DVE ops (2x mode):
                  c1 = [cos | sin] per head, c2 = [sin | cos] per head
                  t3 = qn*c1 -> lo_o = t3.lo - t3.hi
                  t4 = qn*c2 -> hi_o = t4.lo + t4.hi
                """
                qn = qk_nat[i]
                qr = work.tile([128, 5 * HD], F16, tag="qkr", name="qkr")
                qk_r[i] = qr
                t3 = work.tile([128, 5 * HD], F16, tag="t3")
                t4 = work.tile([128, 5 * HD], F16, tag="t4")
                nc.vector.tensor_tensor(t3[:], qn[:], cos_t[:, i, :],
                                        ALU.mult)
                nc.vector.tensor_tensor(t4[:], qn[:], sin_t[:, i, :],
                                        ALU.mult)
                t3v = t3[:].rearrange("p (h d) -> p h d", d=HD)
                t4v = t4[:].rearrange("p (h d) -> p h d", d=HD)
                lo_o = qr[:].rearrange("p (h d) -> p h d", d=HD)[:, :, 0:HALF]
                hi_o = qr[:].rearrange("p (h d) -> p h d", d=HD)[:, :, HALF:HD]
                nc.vector.tensor_tensor(lo_o, t3v[:, :, 0:HALF],
                                        t3v[:, :, HALF:HD], ALU.subtract)
                nc.vector.tensor_tensor(hi_o, t4v[:, :, 0:HALF],
                                        t4v[:, :, HALF:HD], ALU.add)
                # rms scales (q: r; k: r*softmax scale)
                for h in range(G):
                    nc.vector.tensor_scalar(
                        qr[:, h * HD:(h + 1) * HD],
                        qr[:, h * HD:(h + 1) * HD],
                        r_all[:, i * 5 + h:i * 5 + h + 1], None, ALU.mult)
                nc.vector.tensor_scalar(
                    qr[:, 4 * HD:5 * HD], qr[:, 4 * HD:5 * HD],
                    r_all[:, i * 5 + 4:i * 5 + 5], SCALE, ALU.mult, ALU.mult)
                # DMA-xbar transpose into qkT (frees PE + DVE)
                nc.sync.dma_start(qkT[:, :, i * 128:(i + 1) * 128],
                                  qr[:], transpose=True)

            # filler queues: closures emitting one PE-sized unit each
            filler = []

            def pop_filler(n=1):
                for _ in range(n):
                    if filler:
                        filler.pop(0)()

            def queue_proj_chunk(i, tag="a"):
                """Queue projection of chunk i as fine-grained filler.

                The x tile is captured NOW: x_spans slots are reassigned as
                later spans load, so pop-time lookup would read wrong data.
                """
                xt = x_spans[(i // 4) % 2]
                state = {}

                def alloc():
                    state["q"] = ps.tile([128, QCOLS], F32, tag=tag,
                                         name="q_ps")
                    state["kv"] = ps.tile([128, 2 * HD], F32, tag=tag,
                                          name="kv_ps")

                for cc in range(ncc):
                    def unit(cc=cc):
                        if cc == 0:
                            alloc()
                        proj_mms(i, cc, state["q"], state["kv"], xt)
                    filler.append(unit)
                filler.append(lambda: proj_evac(i, state["q"], state["kv"]))
                filler.append(lambda: finish_chunk(i))

            def queue_outproj_quarter(b, j):
                """Queue out-projection of tq chunk (b, j) as filler."""
                ytile = yT[b % 2]
                tq = b * 4 + j
                state = {}

                def evac(cb):
                    o_sb = state["osb"]
                    nc.vector.tensor_copy(
                        o_sb[:, cb * 512:(cb + 1) * 512], state["o"][:])

                for cb in range(C // 512):
                    for hc in range(G):
                        def unit(cb=cb, hc=hc):
                            if cb == 0 and hc == 0:
                                state["osb"] = work.tile(
                                    [128, C], F16, tag="osb", bufs=2,
                                    name="osb")
                            if hc == 0:
                                state["o"] = ps.tile([128, 512], F32,
                                                     tag="a", name="o_ps")
                            nc.tensor.matmul(
                                state["o"][:],
                                ytile[:, hc, j * 128:(j + 1) * 128],
                                wo_t[:, hc, cb * 512:(cb + 1) * 512],
                                start=(hc == 0), stop=(hc == G - 1))
                        filler.append(unit)
                    filler.append(lambda cb=cb: evac(cb))
                filler.append(lambda: nc.sync.dma_start(
                    out[tq * 128:(tq + 1) * 128, :], state["osb"][:]))

            # ---------------- stage B: attention for (head h, block b) -----
            def attn_block(h, b):
                ytile = yT[b % 2]
                # one PSUM bank per j: a matmul with start=True clears
                # has_written for its WHOLE bank, so accumulation groups
                # must not share banks
                y_ps = [ps.tile([128, HD + 1], F32, tag="y", bufs=4,
                                name=f"y{j}") for j in range(4)]
                ydst = [y_ps[j][:] for j in range(4)]
                nk = 4 * (b + 1)
                pts = [None] * nk

                def pv_mms(k):
                    pT = pts[k]
                    for j in range(4):
                        tq = b * 4 + j
                        if tq * 128 < max(b * 512, k * 128):
                            continue
                        loff = tq * 128 - max(b * 512, k * 128)
                        nc.tensor.matmul(
                            ydst[j], pT[:, loff:loff + 128],
                            v_aug[:, k * (HD + 1):(k + 1) * (HD + 1)],
                            start=(k == 0), stop=(k == tq),
                            skip_group_check=True)

                for k in range(nk):
                    tq0 = max(b * 512, k * 128)
                    w = (b + 1) * 512 - tq0
                    diag = k * 128 >= b * 512
                    s_ps = ps.tile([128, 512], F32, tag="s", name="s_ps")
                    nc.tensor.matmul(s_ps[:, 0:w],
                                     qkT[:, 4, k * 128:(k + 1) * 128],
                                     qkT[:, h, tq0:tq0 + w],
                                     start=True, stop=True)
                    pT = ppool.tile([128, 512], F16, tag="pT", name="pT")
                    pts[k] = pT
                    nc.scalar.activation(pT[:, 0:w], s_ps[:, 0:w],
                                         AF.Exp, bias=b_exp[:], scale=1.0)
                    if diag:
                        # zero the tq < tk half of the diagonal block; only
                        # the j == k-4b PV matmul reads these columns.
                        # GPSIMD: near-idle queue, so the exp->mask->PV
                        # chain never waits behind woven DVE rope work
                        nc.gpsimd.tensor_tensor(pT[:, 0:128], pT[:, 0:128],
                                                mask_t[:], ALU.mult)
                    if k > 0:
                        pv_mms(k - 1)
                    pop_filler(1 if k % 2 else 2)
                pv_mms(nk - 1)
                # divide by softmax sums, DMA-xbar transpose into yT block
                y_sb = work.tile([128, 512], F16, tag="ysb")
                for j in range(4):
                    rcp = work.tile([128, 1], F32, tag="rcp")
                    nc.vector.reciprocal(rcp[:], ydst[j][:, HD:HD + 1])
                    nc.vector.tensor_scalar(y_sb[:, j * 128:(j + 1) * 128],
                                            ydst[j][:, 0:HD],
                                            rcp[:], None, ALU.mult)
                nc.sync.dma_start(
                    ytile[:, h, :].rearrange("p (j t) -> p j t", t=128),
                    y_sb[:], transpose=True)

            # ---------------- emission schedule -----------------------------
            # prologue loads spread over three queues, emitted in strict
            # need-order: all pending DMAs share the ~360 GB/s HBM pipe, so
            # anything queued early delays the pieces that gate the first
            # matmuls. cos/sin are per-span pieces for the same reason.
            nc.vector.memset(b_exp[:], EXP_BIAS)
            nc.vector.memset(v_aug[:], 1.0)   # ones cols; data overwritten

            def load_cs_span(s, eng):
                cw = 4 * 5 * HD     # free-dim cols per span
                eng.dma_start(cos_t[:, 4 * s:4 * s + 4, :],
                              cs5[:, s * cw:(s + 1) * cw])
                eng.dma_start(sin_t[:, 4 * s:4 * s + 4, :],
                              sn5[:, s * cw:(s + 1) * cw])

            xt0 = xsp.tile([128, ncc, 512], F16, tag="x", name="x")
            x_spans[0] = xt0
            for q in range(8):   # 2-cc pieces, round-robin across queues
                sl = slice(2 * q, 2 * q + 2)
                nc.gpsimd.dma_start(xt0[:, sl, :],
                                    xtd[0, :, q * 1024:(q + 1) * 1024])
                nc.scalar.dma_start(
                    wq_t[:, sl, :], wq[:, q * 1024:(q + 1) * 1024])
                nc.sync.dma_start(
                    wkv_t[:, sl, :], wkv[:, q * 512:(q + 1) * 512])
            nc.sync.dma_start(mask_t[:], maskd[:])
            load_cs_span(0, nc.scalar)
            if nb > 1:
                load_x_span(1, nc.gpsimd)
            for s in range(1, nb):
                load_cs_span(s, nc.scalar)
            nc.gpsimd.dma_start(wo_t[:], wo[:])

            # prologue compute: project span 0 directly; span 1's chunks are
            # queued as filler so they weave into span-0 attention (the PE
            # keeps streaming while the span-0 DVE rope chain drains).
            # Alternate PSUM tags (the "s" banks are idle pre-attention) so
            # chunk i+1's matmuls never wait on chunk i's PSUM evacuation.
            for i in range(min(4, nt)):
                queue_proj_chunk(i, tag=("s" if i % 2 == 0 else "a"))
            pop_filler(len(filler))
            for i in range(4, min(8, nt)):
                queue_proj_chunk(i)
            if nb > 2:
                load_x_span(2, nc.gpsimd)

            for s in range(nb):
                yT[s % 2] = ypool.tile([128, G, 512], F16, tag="yTb",
                                       name="yTb")
                # queue filler: out-projection of span s-1 + projection of
                # span s+2 (interleaved round-robin-ish: outproj first so
                # yT(s-1) frees early for reuse at span s+1)
                if s > 0:
                    for j in range(G):
                        queue_outproj_quarter(s - 1, j)
                if s + 2 < nb:
                    for i in range(4 * (s + 2), 4 * (s + 3)):
                        queue_proj_chunk(i)
                if s == 0:
                    # span-1 projection chunks keep the PE streaming while
                    # the span-0 rope chain (which gates the first S matmul)
                    # drains on the DVE
                    pop_filler(54)
                for h in range(G):
                    attn_block(h, s)
                    pop_filler(6)
                pop_filler(len(filler))
                # x span s+3 reuses the slot of span s+1, whose last readers
                # (the woven span s+1 projection chunks) were just flushed —
                # emit the load only now so the slot WAR is emission-ordered
                if s + 3 < nb:
                    load_x_span(s + 3, nc.gpsimd)
            for j in range(G):
                queue_outproj_quarter(nb - 1, j)
            pop_filler(len(filler))

    nc.compile()
    return nc


def _pack_rows(a, p=128):
    """[R, N] -> [p, (R//p)*N] partition-major: out[q, c*N+n] = a[c*p+q, n]."""
    R, N = a.shape
    return np.ascontiguousarray(
        a.reshape(R // p, p, N).transpose(1, 0, 2).reshape(p, (R // p) * N))


def _prep_inputs(x, cos, sin, Wq, Wk, Wv, Wo, bo, t):
    """Build the 8 per-core input maps (host-side shard + fp16 cast +
    partition-major packing so device loads are contiguous)."""
    nb_, ncc = t // 512, C // 128
    cos2 = np.asarray(cos, np.float32).reshape(-1, HALF)[:t]
    sin2 = np.asarray(sin, np.float32).reshape(-1, HALF)[:t]
    # interleaved rope tables (see finish_chunk):
    #   c1 = [cos | sin] per head block, c2 = [sin | cos]
    cos5 = np.tile(np.concatenate([cos2, sin2], 1), (1, 5)).astype(np.float16)
    sin5 = np.tile(np.concatenate([sin2, cos2], 1), (1, 5)).astype(np.float16)
    cosp, sinp = _pack_rows(cos5), _pack_rows(sin5)
    mask = np.triu(np.ones((128, 128), np.float16))  # [tk, tq]: 1 iff tq >= tk
    Wq = np.asarray(Wq, np.float32)
    Wk = np.asarray(Wk, np.float32)
    Wv = np.asarray(Wv, np.float32)
    Wo = np.asarray(Wo, np.float32)
    x = np.asarray(x, np.float32)
    maps = []
    for core in range(NC_):
        b, tp = core // TP, core % TP
        wkv = np.concatenate(
            [Wk[:, tp * HD:(tp + 1) * HD], Wv[:, tp * HD:(tp + 1) * HD]],
            axis=1)
        xt = x[b, :t].astype(np.float16).T          # [C, t]
        # [C, t] -> [nb, 128, ncc*512]: xp[s, p, cc*512+tt]=xt[cc*128+p, ...]
        xp = np.ascontiguousarray(
            xt.reshape(ncc, 128, nb_, 512).transpose(2, 1, 0, 3)
            .reshape(nb_, 128, ncc * 512))
        maps.append({
            "xp": xp,
            "wqp": _pack_rows(
                Wq[:, tp * QCOLS:(tp + 1) * QCOLS].astype(np.float16)),
            "wkvp": _pack_rows(wkv.astype(np.float16)),
            "wop": _pack_rows(
                Wo[tp * QCOLS:(tp + 1) * QCOLS, :].astype(np.float16)),
            "cosp": cosp, "sinp": sinp, "mask01": mask,
        })
    return maps


def run(x, cos, sin, Wq, Wk, Wv, Wo, bo, t=T, trace=False):
    key = t
    if key not in _CACHE:
        _CACHE[key] = build_nc(t)
    nc = _CACHE[key]
    maps = _prep_inputs(x, cos, sin, Wq, Wk, Wv, Wo, bo, t)
    res = run_bass_kernel_spmd(nc, maps, core_ids=list(range(NC_)),
                               trace=trace)
    bo = np.asarray(bo, np.float32)
    outp = np.empty((B, t, C), np.float32)
    for b in range(B):
        acc = res.results[b * TP]["out"].astype(np.float32)
        for tp in range(1, TP):
            acc += res.results[b * TP + tp]["out"].astype(np.float32)
        outp[b] = acc + bo[None, :]
    return outp, res


def kernel(x, cos, sin, Wq, Wk, Wv, Wo, bo):
    outp, _ = run(x, cos, sin, Wq, Wk, Wv, Wo, bo, t=T)
    return outp
